# revision 4
# baseline (speedup 1.0000x reference)
"""FSMN BasicBlock (linear -> causal depthwise conv-20 memory + residual ->
affine -> ReLU) Trainium2 Bass kernel.

Contract: kernel(**inputs) takes the FULL unsharded inputs
  input    [16, 2048, 1024] f32
  in_cache [16, 512, 19]    f32
  W_lin    [1024, 512]      f32
  conv_w   [512, 20]        f32
  W_aff    [512, 1024]      f32
  b_aff    [1024]           f32
returns (out [16, 2048, 1024] f32, out_cache [16, 512, 19] f32), matching
the jax reference. Batch is sharded 2 streams per core across 8 NeuronCores
(data-parallel; cache is per-stream state so it shards with batch).

Per-core pipeline:
  - input tiles cast fp32->bf16 in-flight by SWDGE DMA, transposed to
    [d, t] layout by the DMA xbar (2-byte transpose)
  - matmul1 produces x^T (channels on partitions) in PSUM chunks of 512
  - conv taps split across engines: N_PE taps as diag(w_l) matmuls
    accumulating into the same PSUM tile; N_ACT taps scaled on ScalarE;
    N_DVE taps scaled on VectorE (tensor_scalar 4x); all non-PE taps
    added on VectorE (tensor_tensor bf16 2x) into m^T
  - matmul2 uses m^T tiles as the stationary operand so the output lands
    in natural [t, o] layout; ScalarE does ReLU from PSUM; DMA stores
  - out_cache is copied fp32 from the last chunk's x PSUM (no extra
    bf16 rounding)
"""

from contextlib import ExitStack

import numpy as np

import concourse.bass as bass
import concourse.mybir as mybir
from concourse.tile import TileContext
from concourse.masks import make_identity

F32 = mybir.dt.float32
BF16 = mybir.dt.bfloat16
AF = mybir.ActivationFunctionType
ALU = mybir.AluOpType

N_CORES = 8
MAX_DRAIN_WAITS = 1


def _split_excess_waits(nc, max_waits=1):
    """This walrus build accepts very few sync-waits per instruction (the
    Drain lowering takes 1, TensorScalarPtr rejects 2). Hoist excess waits
    from every instruction onto same-engine NoOps inserted just before it —
    engine streams are in-order, so the waits still gate the instruction."""
    n_extra = 0
    for f in nc.m.functions:
        for bb in f.blocks:
            new_insts = []
            for ins in bb.instructions:
                si = ins.sync_info
                waits = list(si.on_wait) if si and si.on_wait else []
                if len(waits) > max_waits:
                    si.on_wait = waits[-max_waits:]
                    rest = waits[: -max_waits]
                    for i in range(0, len(rest), max_waits):
                        nop = mybir.InstNoOp(
                            name=f"{ins.name}_ws{n_extra}",
                            ins=[],
                            outs=[],
                        )
                        nop.engine = ins.engine
                        nop.sync_info = mybir.SyncInfo(
                            on_wait=rest[i : i + max_waits], on_update=[]
                        )
                        nc.register_instruction(nop, overwrite=True)
                        new_insts.append(nop)
                        n_extra += 1
                new_insts.append(ins)
            bb.instructions[:] = new_insts
    return n_extra


def patch_drain():
    """This walrus build only accepts 1 sync-wait on the Drain lowering:
    split the TileContext exit drain's waits across several SP drains."""
    import concourse.tile as tile

    if getattr(tile.TileContext, "_ant_drain_patched", False):
        return

    def _drain_and_barrier(self, tick_clock, wait_clock):
        nc = self.nc
        _split_excess_waits(nc, MAX_DRAIN_WAITS)
        drain_inst = nc.sync.drain()
        wait_clock.add_sem_waits(
            drain_inst.ins, tile.ScopedClock({None: tick_clock.global_clock})
        )
        si = drain_inst.ins.sync_info
        waits = list(si.on_wait or [])
        if len(waits) > MAX_DRAIN_WAITS:
            si.on_wait = waits[:MAX_DRAIN_WAITS]
            rest = waits[MAX_DRAIN_WAITS:]
            for i in range(0, len(rest), MAX_DRAIN_WAITS):
                extra = nc.sync.drain()
                extra.ins.sync_info = mybir.SyncInfo(
                    on_wait=rest[i : i + MAX_DRAIN_WAITS], on_update=[]
                )
        nc.all_engine_barrier()
        assert self.sems is not None
        popped = nc._tile_sem_poison_stack.pop()
        assert popped is self._sem_poison
        nc.clear_and_free_semaphores(list(self.sems.allocated().values()))
        nc.all_engine_barrier()

    tile.TileContext._drain_and_barrier = _drain_and_barrier
    tile.TileContext._ant_drain_patched = True


def build_kernel(
    S=2,           # streams (batches) per core
    T=2048,        # time steps per stream
    D_IN=1024,
    D_P=512,
    D_OUT=1024,
    L=20,          # conv taps
    CHUNK=512,     # psum chunk (<= 512 fp32 psum bank)
    HC=1024,       # conv op granularity along time
    N_PE=6,        # taps on TensorE (diag matmuls)
    N_ACT=2,       # taps scaled on ScalarE, added on VectorE
    N_GP=6,        # taps scaled on GpSimd, added on VectorE
    with_bias=False,
    in_bufs=3,
    inT_bufs=3,
    psum_bufs=3,
    y_bufs=10,
    out_bufs=3,
):
    patch_drain()
    H = L - 1                      # history cols
    KD = D_IN // 128               # k-blocks for matmul1
    PB = D_P // 128                # p-blocks
    NC_CH = T // CHUNK             # chunks per stream
    NHC = T // HC                  # conv blocks per stream
    OH = D_OUT // 512              # o-halves
    TPC = CHUNK // 128             # t-tiles per chunk
    N_DVE = L - N_PE - N_ACT - N_GP
    assert N_DVE >= 0
    PE_TAPS = list(range(N_PE))
    ACT_TAPS = list(range(N_PE, N_PE + N_ACT))
    GP_TAPS = list(range(N_PE + N_ACT, N_PE + N_ACT + N_GP))
    DVE_TAPS = list(range(N_PE + N_ACT + N_GP, L))

    nc = bass.Bass("TRN2")
    x_in = nc.dram_tensor("input", [S, T, D_IN], F32, kind="ExternalInput")
    cache_in = nc.dram_tensor("in_cache", [S, D_P, H], F32, kind="ExternalInput")
    wlin_in = nc.dram_tensor("W_lin", [D_IN, D_P], F32, kind="ExternalInput")
    wconv_in = nc.dram_tensor("conv_w", [D_P, L], F32, kind="ExternalInput")
    waff_in = nc.dram_tensor("W_aff", [D_P, D_OUT], F32, kind="ExternalInput")
    baff_in = nc.dram_tensor("b_aff", [D_OUT], F32, kind="ExternalInput")
    out_t = nc.dram_tensor("out", [S, T, D_OUT], F32, kind="ExternalOutput")
    ocache_t = nc.dram_tensor("out_cache", [S, D_P, H], F32, kind="ExternalOutput")

    with TileContext(nc) as tc, ExitStack() as ctx:
        const = ctx.enter_context(tc.tile_pool(name="const", bufs=1))
        work = ctx.enter_context(tc.tile_pool(name="work", bufs=1))
        inp = ctx.enter_context(tc.tile_pool(name="inp", bufs=in_bufs))
        inTp = ctx.enter_context(tc.tile_pool(name="inTp", bufs=inT_bufs))
        yp = ctx.enter_context(tc.tile_pool(name="yp", bufs=y_bufs))
        outp = ctx.enter_context(tc.tile_pool(name="outp", bufs=out_bufs))
        psum = ctx.enter_context(tc.tile_pool(name="psum", bufs=psum_bufs, space="PSUM"))
        psum_o = ctx.enter_context(tc.tile_pool(name="psum_o", bufs=psum_bufs, space="PSUM"))

        # ---- tiles for weights / constants (DMAs emitted in pipeline order) ----
        wlin_bf = const.tile([128, KD, D_P], BF16)
        waff_bf = const.tile([128, PB, D_OUT], BF16)
        wconv_f = const.tile([128, PB, L], F32)
        xT = [work.tile([128, PB, H + T], BF16, name=f"xT{s}") for s in range(S)]
        mT = [work.tile([128, PB, T], BF16, name=f"mT{s}") for s in range(S)]
        cach = [work.tile([128, PB, H], F32, name=f"cach{s}") for s in range(S)]
        diag = {}

        def emit_wlin_k(k):
            # contiguous 2KB rows -> cheap SWDGE descriptor generation
            nc.gpsimd.dma_start(
                out=wlin_bf[:, k, :],
                in_=wlin_in[k * 128 : (k + 1) * 128, :],
            )

        def emit_waff_k(k):
            nc.gpsimd.dma_start(
                out=waff_bf[:, k, :],
                in_=waff_in[k * 128 : (k + 1) * 128, :],
            )

        def emit_consts():
            nc.sync.dma_start(
                out=wconv_f, in_=wconv_in[:].rearrange("(k p) l -> p k l", p=128)
            )
            if N_PE:
                ident = const.tile([128, 128], BF16)
                make_identity(nc, ident)
                for pb in range(PB):
                    for l in PE_TAPS:
                        d = const.tile([128, 128], BF16, name=f"diag_{pb}_{l}")
                        nc.vector.tensor_scalar_mul(d, ident, wconv_f[:, pb, l : l + 1])
                        diag[(pb, l)] = d
            # cache heads: HWDGE fp32 load + tiny on-chip cast (avoids a
            # 512-descriptor SWDGE generation stall at kernel start)
            for s in range(S):
                cst = const.tile([128, PB, H], F32, name=f"cst{s}")
                nc.sync.dma_start(
                    out=cst, in_=cache_in[s].rearrange("(k p) l -> p k l", p=128)
                )
                nc.vector.tensor_copy(xT[s][:, :, 0:H], cst)

        if with_bias:
            ones_bf = const.tile([1, 128], BF16)
            nc.vector.memset(ones_bf, 1.0)
            brow_bf = const.tile([1, D_OUT], BF16)
            nc.gpsimd.dma_start(out=brow_bf, in_=baff_in[:].rearrange("o -> 1 o"))

        # ---- pipeline stages ----
        loaded = {}

        def emit_load_chunk(s, c):
            in_bf = inp.tile([128, TPC, D_IN], BF16, name="in_bf")
            nc.gpsimd.dma_start(
                out=in_bf,
                in_=x_in[s, c * CHUNK : (c + 1) * CHUNK, :].rearrange(
                    "(i t) d -> t i d", t=128
                ),
            )
            inT = inTp.tile([128, KD, CHUNK], BF16, name="inT")
            for i in range(TPC):
                nc.sync.dma_start_transpose(
                    inT[:, :, i * 128 : (i + 1) * 128], in_bf[:, i, :]
                )
            loaded[(s, c)] = inT

        def emit_mm1_chunk(s, c):
            if (s, c) not in loaded:
                emit_load_chunk(s, c)
            inT = loaded.pop((s, c))
            pxs = []
            for pb in range(PB):
                px = psum.tile([128, CHUNK], F32, name="px")
                for k in range(KD):
                    nc.tensor.matmul(
                        px,
                        lhsT=wlin_bf[:, k, pb * 128 : (pb + 1) * 128],
                        rhs=inT[:, k, :],
                        start=(k == 0),
                        stop=(k == KD - 1),
                    )
                # pure x -> xT (bf16); fp32 tail feeds out_cache
                nc.scalar.copy(xT[s][:, pb, H + c * CHUNK : H + (c + 1) * CHUNK], px)
                if c == NC_CH - 1:
                    nc.scalar.copy(cach[s][:, pb, :], px[:, CHUNK - H : CHUNK])
                pxs.append(px)
            return pxs

        def emit_pe_taps_chunk(s, c, pxs):
            # accumulate PE taps into the x psum, then copy m-partial -> mT
            for pb in range(PB):
                px = pxs[pb]
                for j, l in enumerate(PE_TAPS):
                    nc.tensor.matmul(
                        px,
                        lhsT=diag[(pb, l)],
                        rhs=xT[s][:, pb, c * CHUNK + l : c * CHUNK + l + CHUNK],
                        start=False,
                        stop=(j == N_PE - 1),
                        skip_group_check=True,
                    )
                nc.scalar.copy(mT[s][:, pb, c * CHUNK : (c + 1) * CHUNK], px)

        def emit_conv_block(s, h):
            for pb in range(PB):
                mslice = mT[s][:, pb, h * HC : (h + 1) * HC]
                ys = {}
                # scale producers first (GpSimd / ScalarE run ahead of the
                # VectorE add chain)
                for l in GP_TAPS + ACT_TAPS:
                    y = yp.tile([128, HC], BF16, name="y", tag="y")
                    src = xT[s][:, pb, h * HC + l : h * HC + l + HC]
                    if l in GP_TAPS:
                        nc.gpsimd.tensor_scalar_mul(y, src, wconv_f[:, pb, l : l + 1])
                    else:
                        nc.scalar.activation(
                            y, src, AF.Copy, bias=0.0, scale=wconv_f[:, pb, l : l + 1]
                        )
                    ys[l] = y
                for l in GP_TAPS + ACT_TAPS + DVE_TAPS:
                    if l in DVE_TAPS:
                        y = yp.tile([128, HC], BF16, name="y", tag="y")
                        nc.vector.tensor_scalar_mul(
                            y, xT[s][:, pb, h * HC + l : h * HC + l + HC],
                            wconv_f[:, pb, l : l + 1],
                        )
                    else:
                        y = ys[l]
                    nc.vector.tensor_tensor(mslice, mslice, y, ALU.add)

        def emit_mm2_ttile(s, i):
            ob = outp.tile([128, D_OUT], F32, name="ob")
            for oh in range(OH):
                po = psum_o.tile([128, 512], F32, name="po")
                if with_bias:
                    nc.tensor.matmul(
                        po,
                        lhsT=ones_bf,
                        rhs=brow_bf[:, oh * 512 : (oh + 1) * 512],
                        start=True,
                        stop=False,
                    )
                for pb in range(PB):
                    nc.tensor.matmul(
                        po,
                        lhsT=mT[s][:, pb, i * 128 : (i + 1) * 128],
                        rhs=waff_bf[:, pb, oh * 512 : (oh + 1) * 512],
                        start=(pb == 0 and not with_bias),
                        stop=(pb == PB - 1),
                    )
                nc.scalar.activation(ob[:, oh * 512 : (oh + 1) * 512], po, AF.Relu)
            nc.sync.dma_start(out=out_t[s, i * 128 : (i + 1) * 128, :], in_=ob)

        # Emission order: the first input chunk's DMA goes out before the
        # (descriptor-heavy) weight cast DMAs; streams are interleaved so
        # the VectorE conv chain starts early and mm2 tails overlap; PE
        # taps for chunk c are emitted after mm1 of chunk c+1.
        CPH = HC // CHUNK  # chunks per conv block
        TPH = HC // 128    # t-tiles per conv block

        emit_load_chunk(0, 0)
        for k in range(KD):
            emit_wlin_k(k)
        emit_consts()
        if S > 1:
            emit_load_chunk(1, 0)

        jobs = []
        if S == 2 and NHC == 2:
            jobs += [("mm1", 0, c) for c in range(0, CPH)]
            jobs += [("mm1", 1, c) for c in range(0, CPH)]
            jobs += [("conv", 0, 0)]
            jobs += [("mm1", 0, c) for c in range(CPH, 2 * CPH)]
            jobs += [("conv", 1, 0)]
            jobs += [("mm2", 0, i) for i in range(0, TPH)]
            jobs += [("mm1", 1, c) for c in range(CPH, 2 * CPH)]
            jobs += [("conv", 0, 1)]
            jobs += [("mm2", 1, i) for i in range(0, TPH)]
            jobs += [("conv", 1, 1)]
            jobs += [("mm2", 0, i) for i in range(TPH, 2 * TPH)]
            jobs += [("mm2", 1, i) for i in range(TPH, 2 * TPH)]
        else:
            for s in range(S):
                for h in range(NHC):
                    for c in range(h * CPH, (h + 1) * CPH):
                        jobs.append(("mm1", s, c))
                    jobs.append(("conv", s, h))
                    for i in range(h * TPH, (h + 1) * TPH):
                        jobs.append(("mm2", s, i))

        waff_emitted = 0
        mm1_done = 0
        pending_pe = None
        px_of = {}
        for job in jobs:
            if job[0] == "mm1":
                _, s, c = job
                pxs = emit_mm1_chunk(s, c)
                mm1_done += 1
                # spread the W_aff slice loads behind the first mm1 chunks
                while waff_emitted < PB and mm1_done >= 2 + waff_emitted:
                    emit_waff_k(waff_emitted)
                    waff_emitted += 1
                if pending_pe is not None:
                    emit_pe_taps_chunk(*pending_pe, px_of.pop(pending_pe))
                pending_pe = (s, c)
                px_of[(s, c)] = pxs
            elif job[0] == "conv":
                _, s, h = job
                while waff_emitted < PB:
                    emit_waff_k(waff_emitted)
                    waff_emitted += 1
                if pending_pe is not None:
                    emit_pe_taps_chunk(*pending_pe, px_of.pop(pending_pe))
                    pending_pe = None
                emit_conv_block(s, h)
            else:
                _, s, i = job
                emit_mm2_ttile(s, i)

        for s in range(S):
            nc.sync.dma_start(
                out=ocache_t[s].rearrange("(k p) l -> p k l", p=128), in_=cach[s]
            )

    return nc


_NC_CACHE = {}


def _get_nc(with_bias):
    key = with_bias
    if key not in _NC_CACHE:
        _NC_CACHE[key] = build_kernel(with_bias=with_bias)
    return _NC_CACHE[key]


def kernel(input, in_cache, W_lin, conv_w, W_aff, b_aff, _trace=False, _results=None):
    from concourse.bass_utils import run_bass_kernel_spmd

    input = np.ascontiguousarray(np.asarray(input, dtype=np.float32))
    in_cache = np.ascontiguousarray(np.asarray(in_cache, dtype=np.float32))
    W_lin = np.ascontiguousarray(np.asarray(W_lin, dtype=np.float32))
    conv_w = np.ascontiguousarray(np.asarray(conv_w, dtype=np.float32))
    W_aff = np.ascontiguousarray(np.asarray(W_aff, dtype=np.float32))
    b_aff = np.ascontiguousarray(np.asarray(b_aff, dtype=np.float32))

    B = input.shape[0]
    S = B // N_CORES
    with_bias = bool(np.any(b_aff))
    nc = _get_nc(with_bias)

    in_maps = []
    for c in range(N_CORES):
        sl = slice(c * S, (c + 1) * S)
        in_maps.append(
            {
                "input": np.ascontiguousarray(input[sl]),
                "in_cache": np.ascontiguousarray(in_cache[sl]),
                "W_lin": W_lin,
                "conv_w": conv_w,
                "W_aff": W_aff,
                "b_aff": b_aff,
            }
        )

    res = run_bass_kernel_spmd(
        nc, in_maps, core_ids=list(range(N_CORES)), trace=_trace
    )
    if _results is not None:
        _results.append(res)
    out = np.concatenate([r["out"] for r in res.results], axis=0)
    out_cache = np.concatenate([r["out_cache"] for r in res.results], axis=0)
    return out, out_cache


# revision 5
# speedup vs baseline: 5.0691x; 5.0691x over previous
"""FSMN BasicBlock (linear -> causal depthwise conv-20 memory + residual ->
affine -> ReLU) Trainium2 Bass kernel.

Contract: kernel(**inputs) takes the FULL unsharded inputs
  input    [16, 2048, 1024] f32
  in_cache [16, 512, 19]    f32
  W_lin    [1024, 512]      f32
  conv_w   [512, 20]        f32
  W_aff    [512, 1024]      f32
  b_aff    [1024]           f32
returns (out [16, 2048, 1024] f32, out_cache [16, 512, 19] f32), matching
the jax reference. Batch is sharded 2 streams per core across 8 NeuronCores
(data-parallel; cache is per-stream state so it shards with batch).

Per-core pipeline:
  - input tiles cast fp32->bf16 in-flight by SWDGE DMA, transposed to
    [d, t] layout by the DMA xbar (2-byte transpose)
  - matmul1 produces x^T (channels on partitions) in PSUM chunks of 512
  - conv taps split across engines: N_PE taps as diag(w_l) matmuls
    accumulating into the same PSUM tile; N_ACT taps scaled on ScalarE;
    N_DVE taps scaled on VectorE (tensor_scalar 4x); all non-PE taps
    added on VectorE (tensor_tensor bf16 2x) into m^T
  - matmul2 uses m^T tiles as the stationary operand so the output lands
    in natural [t, o] layout; ScalarE does ReLU from PSUM; DMA stores
  - out_cache is copied fp32 from the last chunk's x PSUM (no extra
    bf16 rounding)
"""

from contextlib import ExitStack

import numpy as np

import concourse.bass as bass
import concourse.mybir as mybir
from concourse.tile import TileContext
from concourse.masks import make_identity

F32 = mybir.dt.float32
BF16 = mybir.dt.bfloat16
AF = mybir.ActivationFunctionType
ALU = mybir.AluOpType

N_CORES = 8
MAX_DRAIN_WAITS = 1


def _split_excess_waits(nc, max_waits=1):
    """This walrus build accepts very few sync-waits per instruction (the
    Drain lowering takes 1, TensorScalarPtr rejects 2). Hoist excess waits
    from every instruction onto same-engine NoOps inserted just before it —
    engine streams are in-order, so the waits still gate the instruction."""
    n_extra = 0
    for f in nc.m.functions:
        for bb in f.blocks:
            new_insts = []
            for ins in bb.instructions:
                si = ins.sync_info
                waits = list(si.on_wait) if si and si.on_wait else []
                if len(waits) > max_waits:
                    si.on_wait = waits[-max_waits:]
                    rest = waits[: -max_waits]
                    for i in range(0, len(rest), max_waits):
                        nop = mybir.InstNoOp(
                            name=f"{ins.name}_ws{n_extra}",
                            ins=[],
                            outs=[],
                        )
                        nop.engine = ins.engine
                        nop.sync_info = mybir.SyncInfo(
                            on_wait=rest[i : i + max_waits], on_update=[]
                        )
                        nc.register_instruction(nop, overwrite=True)
                        new_insts.append(nop)
                        n_extra += 1
                new_insts.append(ins)
            bb.instructions[:] = new_insts
    return n_extra


def patch_drain():
    """This walrus build only accepts 1 sync-wait on the Drain lowering:
    split the TileContext exit drain's waits across several SP drains."""
    import concourse.tile as tile

    if getattr(tile.TileContext, "_ant_drain_patched", False):
        return

    def _drain_and_barrier(self, tick_clock, wait_clock):
        nc = self.nc
        _split_excess_waits(nc, MAX_DRAIN_WAITS)
        drain_inst = nc.sync.drain()
        wait_clock.add_sem_waits(
            drain_inst.ins, tile.ScopedClock({None: tick_clock.global_clock})
        )
        si = drain_inst.ins.sync_info
        waits = list(si.on_wait or [])
        if len(waits) > MAX_DRAIN_WAITS:
            si.on_wait = waits[:MAX_DRAIN_WAITS]
            rest = waits[MAX_DRAIN_WAITS:]
            for i in range(0, len(rest), MAX_DRAIN_WAITS):
                extra = nc.sync.drain()
                extra.ins.sync_info = mybir.SyncInfo(
                    on_wait=rest[i : i + MAX_DRAIN_WAITS], on_update=[]
                )
        nc.all_engine_barrier()
        assert self.sems is not None
        popped = nc._tile_sem_poison_stack.pop()
        assert popped is self._sem_poison
        nc.clear_and_free_semaphores(list(self.sems.allocated().values()))
        nc.all_engine_barrier()

    tile.TileContext._drain_and_barrier = _drain_and_barrier
    tile.TileContext._ant_drain_patched = True


def build_kernel(
    S=2,           # streams (batches) per core
    T=2048,        # time steps per stream
    D_IN=1024,
    D_P=512,
    D_OUT=1024,
    L=20,          # conv taps
    CHUNK=512,     # psum chunk (<= 512 fp32 psum bank)
    HC=1024,       # conv op granularity along time
    N_PE=7,        # taps on TensorE (diag matmuls)
    N_ACT=3,       # taps scaled on ScalarE, added on VectorE
    N_GP=0,        # taps scaled on GpSimd (slow + port-locks DVE: keep 0)
    with_bias=False,
    in_bufs=3,
    inT_bufs=3,
    psum_bufs=3,
    y_bufs=10,
    out_bufs=3,
):
    patch_drain()
    H = L - 1                      # history cols
    KD = D_IN // 128               # k-blocks for matmul1
    PB = D_P // 128                # p-blocks
    NC_CH = T // CHUNK             # chunks per stream
    NHC = T // HC                  # conv blocks per stream
    OH = D_OUT // 512              # o-halves
    TPC = CHUNK // 128             # t-tiles per chunk
    N_DVE = L - N_PE - N_ACT - N_GP
    assert N_DVE >= 0
    PE_TAPS = list(range(N_PE))
    ACT_TAPS = list(range(N_PE, N_PE + N_ACT))
    GP_TAPS = list(range(N_PE + N_ACT, N_PE + N_ACT + N_GP))
    DVE_TAPS = list(range(N_PE + N_ACT + N_GP, L))

    nc = bass.Bass("TRN2")
    x_in = nc.dram_tensor("input", [S, T, D_IN], F32, kind="ExternalInput")
    cache_in = nc.dram_tensor("in_cache", [S, D_P, H], F32, kind="ExternalInput")
    wlin_in = nc.dram_tensor("W_lin", [D_IN, D_P], F32, kind="ExternalInput")
    wconv_in = nc.dram_tensor("conv_w", [D_P, L], F32, kind="ExternalInput")
    waff_in = nc.dram_tensor("W_aff", [D_P, D_OUT], F32, kind="ExternalInput")
    baff_in = nc.dram_tensor("b_aff", [D_OUT], F32, kind="ExternalInput")
    out_t = nc.dram_tensor("out", [S, T, D_OUT], F32, kind="ExternalOutput")
    ocache_t = nc.dram_tensor("out_cache", [S, D_P, H], F32, kind="ExternalOutput")

    with TileContext(nc) as tc, ExitStack() as ctx:
        const = ctx.enter_context(tc.tile_pool(name="const", bufs=1))
        work = ctx.enter_context(tc.tile_pool(name="work", bufs=1))
        inp = ctx.enter_context(tc.tile_pool(name="inp", bufs=in_bufs))
        inTp = ctx.enter_context(tc.tile_pool(name="inTp", bufs=inT_bufs))
        yp = ctx.enter_context(tc.tile_pool(name="yp", bufs=y_bufs))
        outp = ctx.enter_context(tc.tile_pool(name="outp", bufs=out_bufs))
        psum = ctx.enter_context(tc.tile_pool(name="psum", bufs=psum_bufs, space="PSUM"))
        psum_o = ctx.enter_context(tc.tile_pool(name="psum_o", bufs=psum_bufs, space="PSUM"))

        # ---- tiles for weights / constants (DMAs emitted in pipeline order) ----
        wlin_bf = const.tile([128, KD, D_P], BF16)
        waff_bf = const.tile([128, PB, D_OUT], BF16)
        wconv_f = const.tile([128, PB, L], F32)
        xT = [work.tile([128, PB, H + T], BF16, name=f"xT{s}") for s in range(S)]
        mT = [work.tile([128, PB, T], BF16, name=f"mT{s}") for s in range(S)]
        cach = [work.tile([128, PB, H], F32, name=f"cach{s}") for s in range(S)]
        diag = {}

        def emit_wlin_k(k):
            # contiguous 2KB rows -> cheap SWDGE descriptor generation
            nc.gpsimd.dma_start(
                out=wlin_bf[:, k, :],
                in_=wlin_in[k * 128 : (k + 1) * 128, :],
            )

        def emit_waff_k(k):
            nc.gpsimd.dma_start(
                out=waff_bf[:, k, :],
                in_=waff_in[k * 128 : (k + 1) * 128, :],
            )

        def emit_consts():
            nc.sync.dma_start(
                out=wconv_f, in_=wconv_in[:].rearrange("(k p) l -> p k l", p=128)
            )
            if N_PE:
                ident = const.tile([128, 128], BF16)
                make_identity(nc, ident)
                for pb in range(PB):
                    for l in PE_TAPS:
                        d = const.tile([128, 128], BF16, name=f"diag_{pb}_{l}")
                        nc.vector.tensor_scalar_mul(d, ident, wconv_f[:, pb, l : l + 1])
                        diag[(pb, l)] = d
            # cache heads: HWDGE fp32 load + tiny on-chip cast (avoids a
            # 512-descriptor SWDGE generation stall at kernel start)
            for s in range(S):
                cst = const.tile([128, PB, H], F32, name=f"cst{s}")
                nc.sync.dma_start(
                    out=cst, in_=cache_in[s].rearrange("(k p) l -> p k l", p=128)
                )
                nc.vector.tensor_copy(xT[s][:, :, 0:H], cst)

        if with_bias:
            ones_bf = const.tile([1, 128], BF16)
            nc.vector.memset(ones_bf, 1.0)
            brow_bf = const.tile([1, D_OUT], BF16)
            nc.gpsimd.dma_start(out=brow_bf, in_=baff_in[:].rearrange("o -> 1 o"))

        # ---- pipeline stages ----
        loaded = {}

        def emit_load_chunk(s, c):
            in_bf = inp.tile([128, TPC, D_IN], BF16, name="in_bf")
            nc.gpsimd.dma_start(
                out=in_bf,
                in_=x_in[s, c * CHUNK : (c + 1) * CHUNK, :].rearrange(
                    "(i t) d -> t i d", t=128
                ),
            )
            inT = inTp.tile([128, KD, CHUNK], BF16, name="inT")
            for i in range(TPC):
                nc.sync.dma_start_transpose(
                    inT[:, :, i * 128 : (i + 1) * 128], in_bf[:, i, :]
                )
            loaded[(s, c)] = inT

        def emit_mm1_chunk(s, c):
            if (s, c) not in loaded:
                emit_load_chunk(s, c)
            inT = loaded.pop((s, c))
            pxs = []
            for pb in range(PB):
                px = psum.tile([128, CHUNK], F32, name="px")
                for k in range(KD):
                    nc.tensor.matmul(
                        px,
                        lhsT=wlin_bf[:, k, pb * 128 : (pb + 1) * 128],
                        rhs=inT[:, k, :],
                        start=(k == 0),
                        stop=(k == KD - 1),
                    )
                # pure x -> xT (bf16); fp32 tail feeds out_cache
                nc.scalar.copy(xT[s][:, pb, H + c * CHUNK : H + (c + 1) * CHUNK], px)
                if c == NC_CH - 1:
                    nc.scalar.copy(cach[s][:, pb, :], px[:, CHUNK - H : CHUNK])
                pxs.append(px)
            return pxs

        def emit_pe_taps_chunk(s, c, pxs):
            # accumulate PE taps into the x psum, then copy m-partial -> mT
            for pb in range(PB):
                px = pxs[pb]
                for j, l in enumerate(PE_TAPS):
                    nc.tensor.matmul(
                        px,
                        lhsT=diag[(pb, l)],
                        rhs=xT[s][:, pb, c * CHUNK + l : c * CHUNK + l + CHUNK],
                        start=False,
                        stop=(j == N_PE - 1),
                        skip_group_check=True,
                    )
                nc.scalar.copy(mT[s][:, pb, c * CHUNK : (c + 1) * CHUNK], px)

        def emit_conv_block(s, h):
            for pb in range(PB):
                mslice = mT[s][:, pb, h * HC : (h + 1) * HC]
                ys = {}
                # scale producers first (GpSimd / ScalarE run ahead of the
                # VectorE add chain)
                for l in GP_TAPS + ACT_TAPS:
                    y = yp.tile([128, HC], BF16, name="y", tag="y")
                    src = xT[s][:, pb, h * HC + l : h * HC + l + HC]
                    if l in GP_TAPS:
                        nc.gpsimd.tensor_scalar_mul(y, src, wconv_f[:, pb, l : l + 1])
                    else:
                        nc.scalar.activation(
                            y, src, AF.Copy, bias=0.0, scale=wconv_f[:, pb, l : l + 1]
                        )
                    ys[l] = y
                for l in GP_TAPS + ACT_TAPS + DVE_TAPS:
                    if l in DVE_TAPS:
                        y = yp.tile([128, HC], BF16, name="y", tag="y")
                        nc.vector.tensor_scalar_mul(
                            y, xT[s][:, pb, h * HC + l : h * HC + l + HC],
                            wconv_f[:, pb, l : l + 1],
                        )
                    else:
                        y = ys[l]
                    nc.vector.tensor_tensor(mslice, mslice, y, ALU.add)

        def emit_mm2_ttile(s, i):
            ob = outp.tile([128, D_OUT], F32, name="ob")
            for oh in range(OH):
                po = psum_o.tile([128, 512], F32, name="po")
                if with_bias:
                    nc.tensor.matmul(
                        po,
                        lhsT=ones_bf,
                        rhs=brow_bf[:, oh * 512 : (oh + 1) * 512],
                        start=True,
                        stop=False,
                    )
                for pb in range(PB):
                    nc.tensor.matmul(
                        po,
                        lhsT=mT[s][:, pb, i * 128 : (i + 1) * 128],
                        rhs=waff_bf[:, pb, oh * 512 : (oh + 1) * 512],
                        start=(pb == 0 and not with_bias),
                        stop=(pb == PB - 1),
                    )
                nc.scalar.activation(ob[:, oh * 512 : (oh + 1) * 512], po, AF.Relu)
            nc.sync.dma_start(out=out_t[s, i * 128 : (i + 1) * 128, :], in_=ob)

        # Emission order: the first input chunk's DMA goes out before the
        # (descriptor-heavy) weight cast DMAs; streams are interleaved so
        # the VectorE conv chain starts early and mm2 tails overlap; PE
        # taps for chunk c are emitted after mm1 of chunk c+1.
        CPH = HC // CHUNK  # chunks per conv block
        TPH = HC // 128    # t-tiles per conv block

        emit_load_chunk(0, 0)
        for k in range(KD):
            emit_wlin_k(k)
        emit_consts()
        if S > 1:
            emit_load_chunk(1, 0)

        jobs = []
        if S == 2 and NHC == 2:
            jobs += [("mm1", 0, c) for c in range(0, CPH)]
            jobs += [("mm1", 1, c) for c in range(0, CPH)]
            jobs += [("conv", 0, 0)]
            jobs += [("mm1", 0, c) for c in range(CPH, 2 * CPH)]
            jobs += [("conv", 1, 0)]
            jobs += [("mm2", 0, i) for i in range(0, TPH)]
            jobs += [("mm1", 1, c) for c in range(CPH, 2 * CPH)]
            jobs += [("conv", 0, 1)]
            jobs += [("mm2", 1, i) for i in range(0, TPH)]
            jobs += [("conv", 1, 1)]
            jobs += [("mm2", 0, i) for i in range(TPH, 2 * TPH)]
            jobs += [("mm2", 1, i) for i in range(TPH, 2 * TPH)]
        else:
            for s in range(S):
                for h in range(NHC):
                    for c in range(h * CPH, (h + 1) * CPH):
                        jobs.append(("mm1", s, c))
                    jobs.append(("conv", s, h))
                    for i in range(h * TPH, (h + 1) * TPH):
                        jobs.append(("mm2", s, i))

        waff_emitted = 0
        mm1_done = 0
        pending_pe = None
        px_of = {}
        for job in jobs:
            if job[0] == "mm1":
                _, s, c = job
                pxs = emit_mm1_chunk(s, c)
                mm1_done += 1
                # spread the W_aff slice loads behind the first mm1 chunks
                while waff_emitted < PB and mm1_done >= 2 + waff_emitted:
                    emit_waff_k(waff_emitted)
                    waff_emitted += 1
                if pending_pe is not None:
                    emit_pe_taps_chunk(*pending_pe, px_of.pop(pending_pe))
                pending_pe = (s, c)
                px_of[(s, c)] = pxs
            elif job[0] == "conv":
                _, s, h = job
                while waff_emitted < PB:
                    emit_waff_k(waff_emitted)
                    waff_emitted += 1
                if pending_pe is not None:
                    emit_pe_taps_chunk(*pending_pe, px_of.pop(pending_pe))
                    pending_pe = None
                emit_conv_block(s, h)
            else:
                _, s, i = job
                emit_mm2_ttile(s, i)

        for s in range(S):
            nc.sync.dma_start(
                out=ocache_t[s].rearrange("(k p) l -> p k l", p=128), in_=cach[s]
            )

    return nc


_NC_CACHE = {}


def _get_nc(with_bias):
    key = with_bias
    if key not in _NC_CACHE:
        _NC_CACHE[key] = build_kernel(with_bias=with_bias)
    return _NC_CACHE[key]


def kernel(input, in_cache, W_lin, conv_w, W_aff, b_aff, _trace=False, _results=None):
    from concourse.bass_utils import run_bass_kernel_spmd

    input = np.ascontiguousarray(np.asarray(input, dtype=np.float32))
    in_cache = np.ascontiguousarray(np.asarray(in_cache, dtype=np.float32))
    W_lin = np.ascontiguousarray(np.asarray(W_lin, dtype=np.float32))
    conv_w = np.ascontiguousarray(np.asarray(conv_w, dtype=np.float32))
    W_aff = np.ascontiguousarray(np.asarray(W_aff, dtype=np.float32))
    b_aff = np.ascontiguousarray(np.asarray(b_aff, dtype=np.float32))

    B = input.shape[0]
    S = B // N_CORES
    with_bias = bool(np.any(b_aff))
    nc = _get_nc(with_bias)

    in_maps = []
    for c in range(N_CORES):
        sl = slice(c * S, (c + 1) * S)
        in_maps.append(
            {
                "input": np.ascontiguousarray(input[sl]),
                "in_cache": np.ascontiguousarray(in_cache[sl]),
                "W_lin": W_lin,
                "conv_w": conv_w,
                "W_aff": W_aff,
                "b_aff": b_aff,
            }
        )

    res = run_bass_kernel_spmd(
        nc, in_maps, core_ids=list(range(N_CORES)), trace=_trace
    )
    if _results is not None:
        _results.append(res)
    out = np.concatenate([r["out"] for r in res.results], axis=0)
    out_cache = np.concatenate([r["out_cache"] for r in res.results], axis=0)
    return out, out_cache
